# revision 1
# baseline (speedup 1.0000x reference)
"""OFT block-diagonal rotation forward (nn_Linear_12635793785535).

y = x @ blockdiag(rot_0..rot_63), rot_r = I + 2Q_r + 2Q_r^2 + 2Q_r^3 + 2Q_r^4
with Q_r the skew-symmetric matrix built from weight[r].

Sharding: data-parallel over tokens across 8 NeuronCores; the small derived
rotation blocks are replicated (per the problem's sharding hint).

Device kernel per core (1024 tokens, 4096 features), all math exact f32:
  for each 128-token tile:
    DMA x in 4 independent 512KB slabs (compute starts after the first slab)
    for each group of 4 feature-pairs (512 features):
      4x PE transpose x chunks -> one PSUM bank [128, 512]; 1 DVE copy -> SBUF
      4x PE matmul out[tok, feat] = xT.T @ rotpair -> one PSUM bank [128, 512]
      1 ACT copy PSUM -> y slab (DVE keeps the xT copies)
      when a 1024-col y slab completes, DMA it out immediately
Bottleneck (measured): TensorE sequencer; fp32 matmuls are LOW/HIGH 2-pass
with per-pass LDWEIGHTS, ~480ns of PE per 128x128 output tile.
"""

import numpy as np

TOKENS = 8192
FEAT = 4096
R = 64
BLOCK = 64
NPAIR = 32  # pairs of 64-blocks -> 128-wide block-diagonal tiles
GROUP = 4  # pairs per PSUM bank group (4 x 128 = 512 wide)
NGROUP = NPAIR // GROUP  # 8
NUM_TERMS = 5
N_CORES = 8
TOK_SHARD = TOKENS // N_CORES  # 1024
TOK_TILE = 128
N_TTILES = TOK_SHARD // TOK_TILE  # 8

F32R_TRANSPOSE = False

_CACHE = {}

# test.py can flip these before calling kernel()
TRACE = False
LAST_RESULTS = None


def _build_bass():
    from contextlib import ExitStack

    import concourse.tile as tile
    from concourse import bacc, mybir
    from concourse.masks import make_identity

    nc = bacc.Bacc(
        "TRN2",
        target_bir_lowering=False,
        debug=False,
        enable_asserts=False,
        num_devices=N_CORES,
    )
    x_d = nc.dram_tensor(
        "x", [TOK_SHARD, FEAT], mybir.dt.float32, kind="ExternalInput"
    ).ap()
    # rot layout [k=128, pair, c=128]: per-partition contiguous rows for DMA
    rot_d = nc.dram_tensor(
        "rot", [128, NPAIR, 128], mybir.dt.float32, kind="ExternalInput"
    ).ap()
    y_d = nc.dram_tensor(
        "y", [TOK_SHARD, FEAT], mybir.dt.float32, kind="ExternalOutput"
    ).ap()

    with tile.TileContext(nc) as tc, ExitStack() as ctx:
        const_pool = ctx.enter_context(tc.tile_pool(name="const", bufs=1))
        xpool = ctx.enter_context(tc.tile_pool(name="xin", bufs=3))
        ypool = ctx.enter_context(tc.tile_pool(name="yout", bufs=3))
        xtpool = ctx.enter_context(tc.tile_pool(name="xt", bufs=4))
        ps_t = ctx.enter_context(tc.tile_pool(name="ps_t", bufs=3, space="PSUM"))
        ps_y = ctx.enter_context(tc.tile_pool(name="ps_y", bufs=3, space="PSUM"))

        ident = const_pool.tile([128, 128], mybir.dt.float32)
        make_identity(nc, ident)

        rot_sb = const_pool.tile([128, NPAIR, 128], mybir.dt.float32)
        nc.sync.dma_start(rot_sb[:], rot_d)

        SLAB = 1024  # feature columns per DMA slab (2 groups per slab)
        NSLAB = FEAT // SLAB  # 4
        for t in range(N_TTILES):
            tok = slice(t * TOK_TILE, (t + 1) * TOK_TILE)
            # independent slab tiles so compute starts after the first slab
            x_slabs = []
            for s in range(NSLAB):
                xsl = xpool.tile([TOK_TILE, SLAB], mybir.dt.float32, name="xsl", tag="xsl", bufs=6)
                nc.sync.dma_start(xsl[:], x_d[tok, s * SLAB : (s + 1) * SLAB])
                x_slabs.append(xsl)
            y_slabs = [
                ypool.tile([TOK_TILE, SLAB], mybir.dt.float32, name="ysl", tag="ysl", bufs=6)
                for _ in range(NSLAB)
            ]
            for g in range(NGROUP):
                s = g // 2  # slab index; 2 groups per slab
                gc = (g % 2) * GROUP * 128  # column offset within slab
                xt_ps = ps_t.tile([128, GROUP * TOK_TILE], mybir.dt.float32)
                for j in range(GROUP):
                    src = x_slabs[s][:, gc + j * 128 : gc + (j + 1) * 128]
                    dst = xt_ps[:, j * TOK_TILE : (j + 1) * TOK_TILE]
                    nc.tensor.transpose(dst, src, ident[:])
                xt_sb = xtpool.tile([128, GROUP * TOK_TILE], mybir.dt.float32)
                nc.vector.tensor_copy(xt_sb[:], xt_ps[:])
                y_ps = ps_y.tile([TOK_TILE, GROUP * 128], mybir.dt.float32)
                for j in range(GROUP):
                    p = g * GROUP + j
                    nc.tensor.matmul(
                        y_ps[:, j * 128 : (j + 1) * 128],
                        xt_sb[:, j * TOK_TILE : (j + 1) * TOK_TILE],
                        rot_sb[:, p, :],
                        start=True,
                        stop=True,
                    )
                # ACT takes all y copies; DVE keeps the xT copies
                nc.scalar.copy(y_slabs[s][:, gc : gc + GROUP * 128], y_ps[:])
                if g % 2 == 1:
                    # slab complete -> drain it while later groups compute
                    nc.scalar.dma_start(
                        y_d[tok, s * SLAB : (s + 1) * SLAB], y_slabs[s][:]
                    )

    nc.compile()
    return nc


def _host_rot_layout(weight):
    """Cayley-Neumann series on host (f32), packed as [k=128, pair, c=128]
    block-diagonal pair tiles (replicated across cores per sharding hint)."""
    w = np.asarray(weight, dtype=np.float32)
    rows, cols = np.triu_indices(BLOCK, k=1)
    Q = np.zeros((R, BLOCK, BLOCK), dtype=np.float32)
    Q[:, rows, cols] = w
    Q = Q - np.swapaxes(Q, 1, 2)
    eye = np.eye(BLOCK, dtype=np.float32)
    rot = eye[None, :, :] + 2.0 * Q
    Qp = Q
    for _ in range(2, NUM_TERMS):
        Qp = np.einsum("rij,rjk->rik", Qp, Q).astype(np.float32)
        rot = rot + 2.0 * Qp
    layout = np.zeros((128, NPAIR, 128), dtype=np.float32)
    for pair in range(NPAIR):
        layout[0:64, pair, 0:64] = rot[2 * pair]
        layout[64:128, pair, 64:128] = rot[2 * pair + 1]
    return layout


def kernel(x, weight):
    global LAST_RESULTS
    if "nc" not in _CACHE:
        _CACHE["nc"] = _build_bass()
    nc = _CACHE["nc"]

    from concourse.bass_utils import run_bass_kernel_spmd

    x = np.ascontiguousarray(np.asarray(x, dtype=np.float32))
    rot = _host_rot_layout(weight)
    in_maps = [
        {
            "x": np.ascontiguousarray(x[i * TOK_SHARD : (i + 1) * TOK_SHARD]),
            "rot": rot,
        }
        for i in range(N_CORES)
    ]
    res = run_bass_kernel_spmd(
        nc, in_maps, core_ids=list(range(N_CORES)), trace=TRACE
    )
    LAST_RESULTS = res
    out = np.concatenate([r["y"] for r in res.results], axis=0)
    return out



# revision 3
# speedup vs baseline: 1.9939x; 1.9939x over previous
"""OFT block-diagonal rotation forward (nn_Linear_12635793785535).

y = x @ blockdiag(rot_0..rot_63), rot_r = I + 2Q_r + 2Q_r^2 + 2Q_r^3 + 2Q_r^4
with Q_r the skew-symmetric matrix built from weight[r].

Sharding: data-parallel over tokens across 8 NeuronCores; the small derived
rotation blocks are replicated (per the problem's sharding hint).

This kernel is memory-bound (per-core: read 1024x4096 x, write 1024x4096 y).
To halve HBM traffic both streams travel as bf16 (gate is rel_err < 2e-2;
bf16 rounding contributes ~4e-3 absmax/scale). The rotation blocks are
computed exactly in f32 on the host and cast to bf16.

x is transposed on the host so the device runs zero PE transposes:
  xT shard [4096 feat, 1024 tok] bf16; pair p of 64-blocks = rows 128p..128p+127.
Device per pair p (32 pairs):
  DMA in  xT tile [128, 1024] bf16 (one contiguous 256KB transfer)
  2x matmul: lhsT = rot pair tile [128k, 128c] (stationary),
             rhs = xT[:, 512h:512h+512] (moving) -> PSUM yT [128c, 512] f32
  2x copy PSUM f32 -> SBUF bf16 (vector + gpsimd engines)
  DMA out yT tile [128, 1024] bf16
Host reassembles y = concat(yT_core.T).astype(f32).
"""

import numpy as np

TOKENS = 8192
FEAT = 4096
R = 64
BLOCK = 64
NPAIR = 32  # pairs of 64-blocks -> 128-wide block-diagonal tiles
NUM_TERMS = 5
N_CORES = 8
TOK_SHARD = TOKENS // N_CORES  # 1024

_CACHE = {}

# test.py can flip these before calling kernel()
TRACE = False
LAST_RESULTS = None


def _build_bass():
    from contextlib import ExitStack

    import concourse.tile as tile
    from concourse import bacc, mybir

    nc = bacc.Bacc(
        "TRN2",
        target_bir_lowering=False,
        debug=False,
        enable_asserts=False,
        num_devices=N_CORES,
    )
    # xT shard: [feat, tok] so no on-device transpose is needed
    x_d = nc.dram_tensor(
        "x", [FEAT, TOK_SHARD], mybir.dt.bfloat16, kind="ExternalInput"
    ).ap()
    # rot layout [k=128, pair, c=128]: block-diagonal pair tiles
    rot_d = nc.dram_tensor(
        "rot", [128, NPAIR, 128], mybir.dt.bfloat16, kind="ExternalInput"
    ).ap()
    y_d = nc.dram_tensor(
        "y", [FEAT, TOK_SHARD], mybir.dt.bfloat16, kind="ExternalOutput"
    ).ap()

    with tile.TileContext(nc) as tc, ExitStack() as ctx:
        const_pool = ctx.enter_context(tc.tile_pool(name="const", bufs=1))
        xpool = ctx.enter_context(tc.tile_pool(name="xin", bufs=6))
        ypool = ctx.enter_context(tc.tile_pool(name="yout", bufs=6))
        ps_y = ctx.enter_context(tc.tile_pool(name="ps_y", bufs=8, space="PSUM"))

        rot_sb = const_pool.tile([128, NPAIR, 128], mybir.dt.bfloat16)
        nc.sync.dma_start(rot_sb[:], rot_d)

        HALF = TOK_SHARD // 2  # 512 tokens = one PSUM bank of f32
        for p in range(NPAIR):
            row = slice(p * 128, (p + 1) * 128)
            xt = xpool.tile([128, TOK_SHARD], mybir.dt.bfloat16)
            nc.sync.dma_start(xt[:], x_d[row, :])
            yt = ypool.tile([128, TOK_SHARD], mybir.dt.bfloat16)
            for h in range(2):
                ps = ps_y.tile([128, HALF], mybir.dt.float32)
                nc.tensor.matmul(
                    ps[:],
                    rot_sb[:, p, :],
                    xt[:, h * HALF : (h + 1) * HALF],
                    start=True,
                    stop=True,
                )
                if h == 0:
                    nc.vector.tensor_copy(yt[:, h * HALF : (h + 1) * HALF], ps[:])
                else:
                    nc.scalar.copy(yt[:, h * HALF : (h + 1) * HALF], ps[:])
            nc.scalar.dma_start(y_d[row, :], yt[:])

    nc.compile()
    return nc


def _host_rot_layout(weight):
    """Cayley-Neumann series on host (f32), packed as [k=128, pair, c=128]
    block-diagonal pair tiles in bf16 (replicated across cores)."""
    import ml_dtypes

    w = np.asarray(weight, dtype=np.float32)
    rows, cols = np.triu_indices(BLOCK, k=1)
    Q = np.zeros((R, BLOCK, BLOCK), dtype=np.float32)
    Q[:, rows, cols] = w
    Q = Q - np.swapaxes(Q, 1, 2)
    eye = np.eye(BLOCK, dtype=np.float32)
    rot = eye[None, :, :] + 2.0 * Q
    Qp = Q
    for _ in range(2, NUM_TERMS):
        Qp = np.einsum("rij,rjk->rik", Qp, Q).astype(np.float32)
        rot = rot + 2.0 * Qp
    layout = np.zeros((128, NPAIR, 128), dtype=np.float32)
    for pair in range(NPAIR):
        layout[0:64, pair, 0:64] = rot[2 * pair]
        layout[64:128, pair, 64:128] = rot[2 * pair + 1]
    return layout.astype(ml_dtypes.bfloat16)


def kernel(x, weight):
    global LAST_RESULTS
    import ml_dtypes

    if "nc" not in _CACHE:
        _CACHE["nc"] = _build_bass()
    nc = _CACHE["nc"]

    from concourse.bass_utils import run_bass_kernel_spmd

    x = np.asarray(x, dtype=np.float32)
    rot = _host_rot_layout(weight)
    in_maps = [
        {
            # [tok_shard, feat] -> transposed contiguous bf16 [feat, tok_shard]
            "x": x[i * TOK_SHARD : (i + 1) * TOK_SHARD].T.astype(
                ml_dtypes.bfloat16
            ),
            "rot": rot,
        }
        for i in range(N_CORES)
    ]
    res = run_bass_kernel_spmd(
        nc, in_maps, core_ids=list(range(N_CORES)), trace=TRACE
    )
    LAST_RESULTS = res
    out = np.empty((TOKENS, FEAT), dtype=np.float32)
    for i, r in enumerate(res.results):
        out[i * TOK_SHARD : (i + 1) * TOK_SHARD] = r["y"].T
    return out


# revision 4
# speedup vs baseline: 2.1947x; 1.1007x over previous
"""OFT block-diagonal rotation forward (nn_Linear_12635793785535).

y = x @ blockdiag(rot_0..rot_63), rot_r = I + 2Q_r + 2Q_r^2 + 2Q_r^3 + 2Q_r^4
with Q_r the skew-symmetric matrix built from weight[r].

Sharding: data-parallel over tokens across 8 NeuronCores; the small derived
rotation blocks are replicated (per the problem's sharding hint).

This kernel is memory-bound (per-core: read 1024x4096 x, write 1024x4096 y).
Both streams travel as bf16 (gate is rel_err < 2e-2; bf16 rounding
contributes ~7e-3 absmax/scale). Rotation blocks are computed exactly in f32
on the host and cast to bf16.

Layouts are chosen so the device does zero transposes and every DMA is 1 MiB
of 8 KiB-contiguous-per-partition descriptors:
  x_d/y_d: [8 groups, 128 partitions, 4 pairs, 1024 tok] bf16, where
  feature f = g*512 + j*128 + i lives at [g, i, j, :]  (partition-major).
Device per group g (8 groups):
  DMA in  xt [128, 4, 1024] bf16 (1 MiB, sync ring)
  8x matmul: lhsT = rot pair tile [128k, 128c] (stationary),
             rhs = xt[:, j, 512h:512h+512] -> PSUM yT [128c, 512] f32
  8x copy PSUM f32 -> SBUF bf16 (vector/scalar alternate)
  DMA out yt [128, 4, 1024] bf16 (1 MiB, scalar ring)
Host reassembles y from the partition-major layout and upcasts to f32.
"""

import numpy as np

TOKENS = 8192
FEAT = 4096
R = 64
BLOCK = 64
NPAIR = 32  # pairs of 64-blocks -> 128-wide block-diagonal tiles
GROUP = 4  # pairs per DMA group (4 * 256 KiB = 1 MiB)
NGROUP = NPAIR // GROUP  # 8
NUM_TERMS = 5
N_CORES = 8
TOK_SHARD = TOKENS // N_CORES  # 1024

_CACHE = {}

# test.py can flip these before calling kernel()
TRACE = False
LAST_RESULTS = None


def _build_bass():
    from contextlib import ExitStack

    import concourse.tile as tile
    from concourse import bacc, mybir

    nc = bacc.Bacc(
        "TRN2",
        target_bir_lowering=False,
        debug=False,
        enable_asserts=False,
        num_devices=N_CORES,
    )
    x_d = nc.dram_tensor(
        "x", [NGROUP, 128, GROUP, TOK_SHARD], mybir.dt.bfloat16,
        kind="ExternalInput",
    ).ap()
    # rot layout [k=128, pair, c=128]: block-diagonal pair tiles
    rot_d = nc.dram_tensor(
        "rot", [128, NPAIR, 128], mybir.dt.bfloat16, kind="ExternalInput"
    ).ap()
    y_d = nc.dram_tensor(
        "y", [NGROUP, 128, GROUP, TOK_SHARD], mybir.dt.bfloat16,
        kind="ExternalOutput",
    ).ap()

    with tile.TileContext(nc) as tc, ExitStack() as ctx:
        const_pool = ctx.enter_context(tc.tile_pool(name="const", bufs=1))
        xpool = ctx.enter_context(tc.tile_pool(name="xin", bufs=NGROUP))
        ypool = ctx.enter_context(tc.tile_pool(name="yout", bufs=NGROUP))
        ps_y = ctx.enter_context(tc.tile_pool(name="ps_y", bufs=8, space="PSUM"))

        # rot rides the output (scalar/Act) ring, which is idle at start,
        # so x group DMAs on the sync ring begin at t=0.
        rot_sb = const_pool.tile([128, NPAIR, 128], mybir.dt.bfloat16)
        nc.scalar.dma_start(rot_sb[:], rot_d)

        HALF = TOK_SHARD // 2  # 512 tokens = one PSUM bank of f32
        for g in range(NGROUP):
            xt = xpool.tile([128, GROUP, TOK_SHARD], mybir.dt.bfloat16)
            nc.sync.dma_start(xt[:], x_d[g])
            yt = ypool.tile([128, GROUP, TOK_SHARD], mybir.dt.bfloat16)
            for j in range(GROUP):
                p = g * GROUP + j
                for h in range(2):
                    ps = ps_y.tile([128, HALF], mybir.dt.float32)
                    nc.tensor.matmul(
                        ps[:],
                        rot_sb[:, p, :],
                        xt[:, j, h * HALF : (h + 1) * HALF],
                        start=True,
                        stop=True,
                    )
                    dst = yt[:, j, h * HALF : (h + 1) * HALF]
                    if (j * 2 + h) % 2 == 0:
                        nc.vector.tensor_copy(dst, ps[:])
                    else:
                        nc.scalar.copy(dst, ps[:])
            nc.scalar.dma_start(y_d[g], yt[:])

    nc.compile()
    return nc


def _host_rot_layout(weight):
    """Cayley-Neumann series on host (f32), packed as [k=128, pair, c=128]
    block-diagonal pair tiles in bf16 (replicated across cores)."""
    import ml_dtypes

    w = np.asarray(weight, dtype=np.float32)
    rows, cols = np.triu_indices(BLOCK, k=1)
    Q = np.zeros((R, BLOCK, BLOCK), dtype=np.float32)
    Q[:, rows, cols] = w
    Q = Q - np.swapaxes(Q, 1, 2)
    eye = np.eye(BLOCK, dtype=np.float32)
    rot = eye[None, :, :] + 2.0 * Q
    Qp = Q
    for _ in range(2, NUM_TERMS):
        Qp = np.einsum("rij,rjk->rik", Qp, Q).astype(np.float32)
        rot = rot + 2.0 * Qp
    layout = np.zeros((128, NPAIR, 128), dtype=np.float32)
    for pair in range(NPAIR):
        layout[0:64, pair, 0:64] = rot[2 * pair]
        layout[64:128, pair, 64:128] = rot[2 * pair + 1]
    return layout.astype(ml_dtypes.bfloat16)


def kernel(x, weight):
    global LAST_RESULTS
    import ml_dtypes

    if "nc" not in _CACHE:
        _CACHE["nc"] = _build_bass()
    nc = _CACHE["nc"]

    from concourse.bass_utils import run_bass_kernel_spmd

    x = np.asarray(x, dtype=np.float32)
    rot = _host_rot_layout(weight)
    in_maps = []
    for i in range(N_CORES):
        xs = x[i * TOK_SHARD : (i + 1) * TOK_SHARD]  # [1024 tok, 4096 feat]
        # [feat, tok] -> [g, j, i, tok] -> partition-major [g, i, j, tok]
        xg = (
            xs.T.reshape(NGROUP, GROUP, 128, TOK_SHARD)
            .transpose(0, 2, 1, 3)
            .astype(ml_dtypes.bfloat16)
        )
        in_maps.append({"x": xg, "rot": rot})
    res = run_bass_kernel_spmd(
        nc, in_maps, core_ids=list(range(N_CORES)), trace=TRACE
    )
    LAST_RESULTS = res
    out = np.empty((TOKENS, FEAT), dtype=np.float32)
    for i, r in enumerate(res.results):
        yg = r["y"].astype(np.float32)  # [g, i, j, tok]
        out[i * TOK_SHARD : (i + 1) * TOK_SHARD] = (
            yg.transpose(0, 2, 1, 3).reshape(FEAT, TOK_SHARD).T
        )
    return out
